# revision 6
# baseline (speedup 1.0000x reference)
"""MoE layer (top-2 of 8 experts, H=1024, FFN=4096) on 8 TRN2 NeuronCores.

Expert-parallel: core e holds expert e's weights. The (tiny) router runs on
host; tokens are gathered per-expert into capacity-padded batches, each core
runs the expert FFN (x @ w1.T -> +b1 -> gelu -> @ w2.T -> +b2 -> *gate) and
the host scatter-adds the two expert contributions per token back together.

Device layout per core (C = per-expert token capacity, multiple of 128):
  GEMM1  h[f, c] = w1t[h, f].T @ xT[h, c]   (F on partitions, tokens free)
  GEMM2  y[c, n] = h[f, c].T  @ w2t[f, n]   (tokens on partitions, H free)
b1 is per-partition in GEMM1's output (activation bias), b2 is broadcast
along partitions via a ones-row matmul trick, gate is per-partition in
GEMM2's output.
"""

import numpy as np

import concourse.bass as bass  # noqa: F401  (bass types via bacc)
import concourse.mybir as mybir
from concourse import bacc
from concourse.tile import TileContext
from concourse.bass_utils import run_bass_kernel_spmd

H = 1024
E = 8
F = 4096
TOPK = 2
P = 128
N_CORES = 8
FP32 = mybir.dt.float32

_cache: dict = {}

# Test-harness knobs (harness-safe defaults): set TRACE=True before calling
# kernel() to profile the device run; exec time lands in LAST_EXEC_TIME_NS.
TRACE = False
LAST_EXEC_TIME_NS = None


def _build(C: int):
    """Build + compile the per-core expert-FFN program for capacity C."""
    assert C % P == 0
    n_ct = C // P
    cbs = []
    off = 0
    while off < C:
        w = min(512, C - off)
        cbs.append((off, w))
        off += w

    NTH = 8            # number of F slabs ("eighths")
    FT = F // NTH      # 512 F columns per slab
    MF = FT // P       # 4 m-tiles of 128 per slab
    KH = H // P        # 8 contraction tiles for GEMM1

    nc = bacc.Bacc("TRN2", target_bir_lowering=False, debug=False,
                   num_devices=N_CORES)

    xT = nc.dram_tensor("xT", [H, C], FP32, kind="ExternalInput")
    w1t = nc.dram_tensor("w1t", [H, F], FP32, kind="ExternalInput")
    w2t = nc.dram_tensor("w2t", [F, H], FP32, kind="ExternalInput")
    b1c = nc.dram_tensor("b1c", [P, F // P], FP32, kind="ExternalInput")
    b2r = nc.dram_tensor("b2r", [1, H], FP32, kind="ExternalInput")
    gate = nc.dram_tensor("gate", [P, n_ct], FP32, kind="ExternalInput")
    out = nc.dram_tensor("out", [C, H], FP32, kind="ExternalOutput")

    xT_v = xT.rearrange("(k p) c -> p k c", p=P)     # [128, 8, C]
    w1_v = w1t.rearrange("(k p) f -> p k f", p=P)    # [128, 8, F]
    w2_v = w2t.rearrange("(f p) h -> p f h", p=P)    # [128, 32, H]
    out_v = out.rearrange("(j p) h -> p j h", p=P)   # [128, n_ct, H]

    GELU = mybir.ActivationFunctionType.Gelu
    ADD = mybir.AluOpType.add

    with TileContext(nc) as tc:
        with (
            tc.tile_pool(name="const", bufs=1) as constp,
            tc.tile_pool(name="xp", bufs=1) as xp,
            tc.tile_pool(name="w1p", bufs=2) as w1p,
            tc.tile_pool(name="w2p", bufs=2) as w2p,
            tc.tile_pool(name="hp", bufs=3) as hp,
            tc.tile_pool(name="yp", bufs=1) as yp,
            tc.tile_pool(name="op", bufs=2) as op,
            tc.tile_pool(name="ps", bufs=4, space="PSUM") as ps,
        ):
            b1_sb = constp.tile([P, F // P], FP32, tag="b1")
            nc.sync.dma_start(out=b1_sb[:], in_=b1c[:])
            gate_sb = constp.tile([P, n_ct], FP32, tag="gate")
            nc.sync.dma_start(out=gate_sb[:], in_=gate[:])

            # b2 broadcast across partitions: psum = onesrow.T @ b2row
            b2row = constp.tile([P, H], FP32, tag="b2row")
            nc.vector.memset(b2row[:], 0.0)
            nc.sync.dma_start(out=b2row[0:1, :], in_=b2r[:])
            ones_t = constp.tile([P, P], FP32, tag="ones")
            nc.vector.memset(ones_t[:], 0.0)
            nc.vector.memset(ones_t[0:1, :], 1.0)
            b2bc = constp.tile([P, H], FP32, tag="b2bc")
            for n2 in range(2):
                pt = ps.tile([P, 512], FP32, tag="ps2", name=f"psb2_{n2}")
                nc.tensor.matmul(pt[:], ones_t[:], b2row[:, n2 * 512:(n2 + 1) * 512],
                                 start=True, stop=True)
                nc.scalar.copy(b2bc[:, n2 * 512:(n2 + 1) * 512], pt[:])

            x_sb = []
            for k in range(KH):
                t = xp.tile([P, C], FP32, tag=f"x{k}", name=f"x{k}")
                nc.sync.dma_start(out=t[:], in_=xT_v[:, k, :])
                x_sb.append(t)

            y_sb = [yp.tile([P, H], FP32, tag=f"y{j}", name=f"y{j}") for j in range(n_ct)]

            for th in range(NTH):
                w1_t = []
                for k in range(KH):
                    t = w1p.tile([P, FT], FP32, tag=f"w1k{k}", name=f"w1_{th}_{k}")
                    nc.sync.dma_start(out=t[:], in_=w1_v[:, k, th * FT:(th + 1) * FT])
                    w1_t.append(t)
                w2_t = []
                for m in range(MF):
                    t = w2p.tile([P, H], FP32, tag=f"w2m{m}", name=f"w2_{th}_{m}")
                    nc.sync.dma_start(out=t[:], in_=w2_v[:, th * MF + m, :])
                    w2_t.append(t)

                for (coff, cw) in cbs:
                    h_t = hp.tile([P, MF, cw], FP32, tag="h")
                    for m in range(MF):
                        pt = ps.tile([P, cw], FP32, tag="ps1")
                        for k in range(KH):
                            nc.tensor.matmul(
                                pt[:],
                                w1_t[k][:, m * P:(m + 1) * P],
                                x_sb[k][:, coff:coff + cw],
                                start=(k == 0), stop=(k == KH - 1),
                            )
                        nc.scalar.activation(
                            h_t[:, m, :], pt[:], GELU,
                            bias=b1_sb[:, th * MF + m:th * MF + m + 1],
                        )
                    for ct in range(cw // P):
                        j = (coff // P) + ct
                        for n2 in range(2):
                            pt2 = ps.tile([P, 512], FP32, tag="ps2")
                            for m in range(MF):
                                nc.tensor.matmul(
                                    pt2[:],
                                    h_t[:, m, ct * P:(ct + 1) * P],
                                    w2_t[m][:, n2 * 512:(n2 + 1) * 512],
                                    start=(m == 0), stop=(m == MF - 1),
                                )
                            ys = y_sb[j][:, n2 * 512:(n2 + 1) * 512]
                            if th == 0:
                                nc.vector.tensor_copy(ys, pt2[:])
                            else:
                                nc.vector.tensor_tensor(ys, ys, pt2[:], ADD)

            for j in range(n_ct):
                o_t = op.tile([P, H], FP32, tag="o")
                nc.vector.tensor_tensor(o_t[:], y_sb[j][:], b2bc[:], ADD)
                nc.vector.tensor_scalar_mul(o_t[:], o_t[:], gate_sb[:, j:j + 1])
                nc.sync.dma_start(out=out_v[:, j, :], in_=o_t[:])

    nc.compile()
    return nc


def _route(x: np.ndarray, router_w: np.ndarray):
    """Host router: top-2 expert ids + softmax gates per token."""
    logits = x @ router_w.T                                   # [T, E]
    top_i = np.argsort(-logits, axis=1, kind="stable")[:, :TOPK]
    top_v = np.take_along_axis(logits, top_i, axis=1)
    mx = top_v.max(axis=1, keepdims=True)
    ex = np.exp(top_v - mx)
    rw = ex / ex.sum(axis=1, keepdims=True)
    return top_i, rw.astype(np.float32)


def kernel(hidden_states, router_w, w1, b1, w2, b2):
    hidden_states = np.ascontiguousarray(np.asarray(hidden_states, np.float32))
    router_w = np.ascontiguousarray(np.asarray(router_w, np.float32))
    w1 = np.asarray(w1, np.float32)
    b1 = np.asarray(b1, np.float32)
    w2 = np.asarray(w2, np.float32)
    b2 = np.asarray(b2, np.float32)

    B, S, _ = hidden_states.shape
    T = B * S
    x = hidden_states.reshape(T, H)

    top_i, rw = _route(x, router_w)

    sel_idx = []
    sel_gate = []
    for e in range(E):
        mask = top_i == e                                     # [T, K]
        rows = np.nonzero(mask.any(axis=1))[0]
        g = rw[rows[:, None], np.argmax(mask[rows], axis=1)[:, None]][:, 0]
        sel_idx.append(rows)
        sel_gate.append(g.astype(np.float32))

    cmax = max(len(r) for r in sel_idx)
    C = max(P, int(-(-cmax // P)) * P)

    key = C
    if key not in _cache:
        _cache[key] = _build(C)
    nc = _cache[key]

    in_maps = []
    for e in range(E):
        n_e = len(sel_idx[e])
        xT_e = np.zeros((H, C), np.float32)
        xT_e[:, :n_e] = x[sel_idx[e]].T
        gate_e = np.zeros(C, np.float32)
        gate_e[:n_e] = sel_gate[e]
        in_maps.append({
            "xT": xT_e,
            "w1t": np.ascontiguousarray(w1[e].T),
            "w2t": np.ascontiguousarray(w2[e].T),
            "b1c": np.ascontiguousarray(b1[e].reshape(F // P, P).T),
            "b2r": np.ascontiguousarray(b2[e].reshape(1, H)),
            "gate": np.ascontiguousarray(gate_e.reshape(C // P, P).T),
        })

    res = run_bass_kernel_spmd(nc, in_maps, list(range(N_CORES)), trace=TRACE)
    global LAST_EXEC_TIME_NS
    LAST_EXEC_TIME_NS = res.exec_time_ns

    out = np.zeros((T, H), np.float32)
    for e in range(E):
        n_e = len(sel_idx[e])
        if n_e:
            # row indices are unique within one expert, so += is safe
            out[sel_idx[e]] += res.results[e]["out"][:n_e]

    return out.reshape(B, S, H)


# revision 7
# speedup vs baseline: 3.5822x; 3.5822x over previous
"""MoE layer (top-2 of 8 experts, H=1024, FFN=4096) on 8 TRN2 NeuronCores.

Expert-parallel: core e holds expert e's weights. The (tiny) router runs on
host; tokens are gathered per-expert into capacity-padded batches, each core
runs the expert FFN (x @ w1.T -> +b1 -> gelu -> @ w2.T -> +b2 -> *gate) and
the host scatter-adds the two expert contributions per token back together.

Device layout per core (C = per-expert token capacity, multiple of 128):
  GEMM1  h[f, c] = w1t[h, f].T @ xT[h, c]   (F on partitions, tokens free)
  GEMM2  y[c, n] = h[f, c].T  @ w2t[f, n]   (tokens on partitions, H free)
b1 is per-partition in GEMM1's output (activation bias), b2 is broadcast
along partitions via a ones-row matmul trick, gate is per-partition in
GEMM2's output.
"""

import ml_dtypes
import numpy as np

import concourse.bass as bass  # noqa: F401  (bass types via bacc)
import concourse.mybir as mybir
from concourse import bacc
from concourse.tile import TileContext
from concourse.bass_utils import run_bass_kernel_spmd

H = 1024
E = 8
F = 4096
TOPK = 2
P = 128
N_CORES = 8
FP32 = mybir.dt.float32
BF16 = mybir.dt.bfloat16

_cache: dict = {}

# Test-harness knobs (harness-safe defaults): set TRACE=True before calling
# kernel() to profile the device run; exec time lands in LAST_EXEC_TIME_NS.
TRACE = False
LAST_EXEC_TIME_NS = None


def _build(C: int):
    """Build + compile the per-core expert-FFN program for capacity C."""
    assert C % P == 0
    n_ct = C // P
    cbs = []
    off = 0
    while off < C:
        w = min(512, C - off)
        cbs.append((off, w))
        off += w

    NTH = 8            # number of F slabs ("eighths")
    FT = F // NTH      # 512 F columns per slab
    MF = FT // P       # 4 m-tiles of 128 per slab
    KH = H // P        # 8 contraction tiles for GEMM1

    nc = bacc.Bacc("TRN2", target_bir_lowering=False, debug=False,
                   num_devices=N_CORES)

    xT = nc.dram_tensor("xT", [H, C], BF16, kind="ExternalInput")
    w1t = nc.dram_tensor("w1t", [H, F], BF16, kind="ExternalInput")
    w2t = nc.dram_tensor("w2t", [F, H], BF16, kind="ExternalInput")
    b1c = nc.dram_tensor("b1c", [P, F // P], FP32, kind="ExternalInput")
    b2r = nc.dram_tensor("b2r", [1, H], FP32, kind="ExternalInput")
    gate = nc.dram_tensor("gate", [P, n_ct], FP32, kind="ExternalInput")
    out = nc.dram_tensor("out", [C, H], FP32, kind="ExternalOutput")

    xT_v = xT.rearrange("(k p) c -> p k c", p=P)     # [128, 8, C]
    w1_v = w1t.rearrange("(k p) f -> p k f", p=P)    # [128, 8, F]
    w2_v = w2t.rearrange("(f p) h -> p f h", p=P)    # [128, 32, H]
    out_v = out.rearrange("(j p) h -> p j h", p=P)   # [128, n_ct, H]

    GELU = mybir.ActivationFunctionType.Gelu
    ADD = mybir.AluOpType.add

    with TileContext(nc) as tc:
        with (
            tc.tile_pool(name="const", bufs=1) as constp,
            tc.tile_pool(name="xp", bufs=1) as xp,
            tc.tile_pool(name="w1p", bufs=2) as w1p,
            tc.tile_pool(name="w2p", bufs=2) as w2p,
            tc.tile_pool(name="hp", bufs=3) as hp,
            tc.tile_pool(name="yp", bufs=1) as yp,
            tc.tile_pool(name="op", bufs=2) as op,
            tc.tile_pool(name="ps", bufs=4, space="PSUM") as ps,
        ):
            b1_sb = constp.tile([P, F // P], FP32, tag="b1")
            nc.sync.dma_start(out=b1_sb[:], in_=b1c[:])
            gate_sb = constp.tile([P, n_ct], FP32, tag="gate")
            nc.sync.dma_start(out=gate_sb[:], in_=gate[:])

            # b2 broadcast across partitions: psum = onesrow.T @ b2row
            b2row = constp.tile([P, H], FP32, tag="b2row")
            nc.vector.memset(b2row[:], 0.0)
            nc.sync.dma_start(out=b2row[0:1, :], in_=b2r[:])
            ones_t = constp.tile([P, P], FP32, tag="ones")
            nc.vector.memset(ones_t[:], 0.0)
            nc.vector.memset(ones_t[0:1, :], 1.0)
            b2bc = constp.tile([P, H], FP32, tag="b2bc")
            for n2 in range(2):
                pt = ps.tile([P, 512], FP32, tag="ps2", name=f"psb2_{n2}")
                nc.tensor.matmul(pt[:], ones_t[:], b2row[:, n2 * 512:(n2 + 1) * 512],
                                 start=True, stop=True)
                nc.scalar.copy(b2bc[:, n2 * 512:(n2 + 1) * 512], pt[:])

            x_sb = []
            for k in range(KH):
                t = xp.tile([P, C], BF16, tag=f"x{k}", name=f"x{k}")
                nc.sync.dma_start(out=t[:], in_=xT_v[:, k, :])
                x_sb.append(t)

            y_sb = [yp.tile([P, H], FP32, tag=f"y{j}", name=f"y{j}") for j in range(n_ct)]

            for th in range(NTH):
                w1_t = []
                for k in range(KH):
                    t = w1p.tile([P, FT], BF16, tag=f"w1k{k}", name=f"w1_{th}_{k}")
                    nc.sync.dma_start(out=t[:], in_=w1_v[:, k, th * FT:(th + 1) * FT])
                    w1_t.append(t)
                w2_t = []
                for m in range(MF):
                    t = w2p.tile([P, H], BF16, tag=f"w2m{m}", name=f"w2_{th}_{m}")
                    nc.sync.dma_start(out=t[:], in_=w2_v[:, th * MF + m, :])
                    w2_t.append(t)

                for (coff, cw) in cbs:
                    h_t = hp.tile([P, MF, cw], BF16, tag="h")
                    for m in range(MF):
                        pt = ps.tile([P, cw], FP32, tag="ps1")
                        for k in range(KH):
                            nc.tensor.matmul(
                                pt[:],
                                w1_t[k][:, m * P:(m + 1) * P],
                                x_sb[k][:, coff:coff + cw],
                                start=(k == 0), stop=(k == KH - 1),
                            )
                        nc.scalar.activation(
                            h_t[:, m, :], pt[:], GELU,
                            bias=b1_sb[:, th * MF + m:th * MF + m + 1],
                        )
                    for ct in range(cw // P):
                        j = (coff // P) + ct
                        for n2 in range(2):
                            pt2 = ps.tile([P, 512], FP32, tag="ps2")
                            for m in range(MF):
                                nc.tensor.matmul(
                                    pt2[:],
                                    h_t[:, m, ct * P:(ct + 1) * P],
                                    w2_t[m][:, n2 * 512:(n2 + 1) * 512],
                                    start=(m == 0), stop=(m == MF - 1),
                                )
                            ys = y_sb[j][:, n2 * 512:(n2 + 1) * 512]
                            if th == 0:
                                nc.vector.tensor_copy(ys, pt2[:])
                            else:
                                nc.vector.tensor_tensor(ys, ys, pt2[:], ADD)

            for j in range(n_ct):
                o_t = op.tile([P, H], FP32, tag="o")
                nc.vector.tensor_tensor(o_t[:], y_sb[j][:], b2bc[:], ADD)
                nc.vector.tensor_scalar_mul(o_t[:], o_t[:], gate_sb[:, j:j + 1])
                nc.sync.dma_start(out=out_v[:, j, :], in_=o_t[:])

    nc.compile()
    return nc


def _route(x: np.ndarray, router_w: np.ndarray):
    """Host router: top-2 expert ids + softmax gates per token."""
    logits = x @ router_w.T                                   # [T, E]
    top_i = np.argsort(-logits, axis=1, kind="stable")[:, :TOPK]
    top_v = np.take_along_axis(logits, top_i, axis=1)
    mx = top_v.max(axis=1, keepdims=True)
    ex = np.exp(top_v - mx)
    rw = ex / ex.sum(axis=1, keepdims=True)
    return top_i, rw.astype(np.float32)


def kernel(hidden_states, router_w, w1, b1, w2, b2):
    hidden_states = np.ascontiguousarray(np.asarray(hidden_states, np.float32))
    router_w = np.ascontiguousarray(np.asarray(router_w, np.float32))
    w1 = np.asarray(w1, np.float32)
    b1 = np.asarray(b1, np.float32)
    w2 = np.asarray(w2, np.float32)
    b2 = np.asarray(b2, np.float32)

    B, S, _ = hidden_states.shape
    T = B * S
    x = hidden_states.reshape(T, H)

    top_i, rw = _route(x, router_w)

    sel_idx = []
    sel_gate = []
    for e in range(E):
        mask = top_i == e                                     # [T, K]
        rows = np.nonzero(mask.any(axis=1))[0]
        g = rw[rows[:, None], np.argmax(mask[rows], axis=1)[:, None]][:, 0]
        sel_idx.append(rows)
        sel_gate.append(g.astype(np.float32))

    cmax = max(len(r) for r in sel_idx)
    C = max(P, int(-(-cmax // P)) * P)

    key = C
    if key not in _cache:
        _cache[key] = _build(C)
    nc = _cache[key]

    in_maps = []
    for e in range(E):
        n_e = len(sel_idx[e])
        xT_e = np.zeros((H, C), ml_dtypes.bfloat16)
        xT_e[:, :n_e] = x[sel_idx[e]].T.astype(ml_dtypes.bfloat16)
        gate_e = np.zeros(C, np.float32)
        gate_e[:n_e] = sel_gate[e]
        in_maps.append({
            "xT": xT_e,
            "w1t": np.ascontiguousarray(w1[e].T).astype(ml_dtypes.bfloat16),
            "w2t": np.ascontiguousarray(w2[e].T).astype(ml_dtypes.bfloat16),
            "b1c": np.ascontiguousarray(b1[e].reshape(F // P, P).T),
            "b2r": np.ascontiguousarray(b2[e].reshape(1, H)),
            "gate": np.ascontiguousarray(gate_e.reshape(C // P, P).T),
        })

    res = run_bass_kernel_spmd(nc, in_maps, list(range(N_CORES)), trace=TRACE)
    global LAST_EXEC_TIME_NS
    LAST_EXEC_TIME_NS = res.exec_time_ns

    out = np.zeros((T, H), np.float32)
    for e in range(E):
        n_e = len(sel_idx[e])
        if n_e:
            # row indices are unique within one expert, so += is safe
            out[sel_idx[e]] += res.results[e]["out"][:n_e]

    return out.reshape(B, S, H)
